# revision 8
# baseline (speedup 1.0000x reference)
"""GrowingCrystalAttention Trainium2 kernel.

Expert-parallel over 8 NeuronCores: each core handles 16 of the 128
"neurons" (experts). Per core:
  - attention: xp = X @ posT (bf16 PE), dist/softmax on ACT+DVE in fp32
  - main contraction, tile-outer: for each of 16 bt tiles, P_n = X @ W'_n
    as bf16 matmuls where W'_n = W_n @ out_W.T is folded on the host,
    acc += attn[:, n] * P_n via DVE stt / ACT scale + GPSIMD add.
    Experts are processed in groups of 4 sharing the stationary X chunk;
    when LDW_SKIP is on, the 3 trailing matmuls of each (group, k) skip
    the redundant LDWEIGHTS (explicit PE-order chain keeps this sound).
  - out_b/8 is folded into the first drain of each tile, so after the
    ReduceScatter the shard is final; partials RS'd in 4 chunks, the
    last chunk tiny so only ~a collective floor is exposed at the end
  - all output copies (rs_out -> y) happen at the very end so they never
    block partial-DMA dispatch (that serialized the collectives before)
  - weights live resident in SBUF (8.4 MB bf16), loaded once in 4 DMAs

SPMD trick: every core runs the identical program; per-core inputs are
permuted so that attention columns 0..15 are always the core's own experts.
"""
import os
import sys

sys.path.insert(0, "/opt/trn_rl_repo")

import numpy as np
import ml_dtypes

import concourse.bass as bass
import concourse.mybir as mybir
import concourse.tile as tile
from concourse import bacc
from concourse.bass import ts
from concourse.bass_utils import run_bass_kernel_spmd
from concourse.tile_rust import add_dep_helper

AF = mybir.ActivationFunctionType
F32 = mybir.dt.float32
BF16 = mybir.dt.bfloat16

NCORES = 8
B, T, D = 4, 512, 512
N = 128
BT = B * T          # 2048
NLOC = N // NCORES  # 16
NTILES = BT // 128  # 16
KCH = D // 128      # 4
# bt-tile blocks; each ends with a ReduceScatter.  Last chunk is tiny so
# only its (small) collective is exposed after the final matmul.
BLKS = [(0, 6), (6, 12), (12, 15), (15, 16)]
GEXP = 4                      # experts per group (shared stationary)
WCOL = KCH * D                # weight columns per expert in wall
GCOL = GEXP * WCOL            # weight columns per DMA group
LDW_SKIP = True               # skip LDWEIGHTS on trailing group matmuls

_PROGRAM = None  # cached across kernel() calls


def _mm(nc, out, lhsT, rhs, start, stop, ldw, prev):
    """matmul with explicit ldweights control + PE-order chaining."""
    if ldw:
        inst = nc.tensor.matmul(out, lhsT, rhs, start=start, stop=stop)
    else:
        eng = nc.tensor
        ifmap_ap = eng.lower_ap(rhs.opt({0}), opt=False)
        weights_ap = eng.lower_ap(lhsT.opt({0}), opt=False,
                                  for_matmul_weights=True)
        out_ap = eng.lower_ap(out)
        inst = eng.add_instruction(mybir.InstMatmult(
            name=eng.bass.get_next_instruction_name(),
            replication_resolution=0, replication_shift_amnt=0,
            replication_num_rows=0,
            start_tensor_calc=start, stop_tensor_calc=stop,
            ins=[ifmap_ap, weights_ap], outs=[out_ap],
            tile_position=(0, 0),
            tile_size=(128, 128),
            ldweights=False,
        ))
    if prev is not None:
        add_dep_helper(inst.ins, prev.ins, sync=True, reason="pe-order")
    return inst


def _build_program():
    nc = bacc.Bacc("TRN2", target_bir_lowering=False, debug=False,
                   num_devices=NCORES)

    xt_h = nc.dram_tensor("xt_h", [D, BT], BF16, kind="ExternalInput").ap()
    x2m = nc.dram_tensor("x2m", [128, NTILES], F32, kind="ExternalInput").ap()
    post = nc.dram_tensor("post", [D, N], BF16, kind="ExternalInput").ap()
    aug = nc.dram_tensor("aug", [1, N], BF16, kind="ExternalInput").ap()
    scb = nc.dram_tensor("scb", [128, N], F32, kind="ExternalInput").ap()
    wall = nc.dram_tensor("wall", [128, NLOC * WCOL], BF16,
                          kind="ExternalInput").ap()
    obb = nc.dram_tensor("obb", [128, D], F32, kind="ExternalInput").ap()
    y = nc.dram_tensor("y", [BT // NCORES, D], F32, kind="ExternalOutput").ap()

    with tile.TileContext(nc) as tc:
        with tc.tile_pool(name="const", bufs=1) as constp, \
             tc.tile_pool(name="tmp", bufs=4) as tmpp, \
             tc.tile_pool(name="stat", bufs=6) as statp, \
             tc.tile_pool(name="pmain", bufs=6, space="PSUM") as pmain, \
             tc.tile_pool(name="psmall", bufs=2, space="PSUM") as psmall, \
             tc.tile_pool(name="dram", bufs=1, space="DRAM") as dramp:

            # ---- input DMAs on the sync queue, in priority order ----
            xth = [constp.tile([128, BT], BF16, tag=f"xth{k}", name=f"xth{k}")
                   for k in range(KCH)]
            for k in range(KCH):
                nc.sync.dma_start(xth[k][:], xt_h[ts(k, 128), :])
            postt = [constp.tile([128, N], BF16, tag=f"post{k}", name=f"post{k}")
                     for k in range(KCH)]
            for k in range(KCH):
                nc.sync.dma_start(postt[k][:], post[ts(k, 128), :])
            wallt = constp.tile([128, NLOC * WCOL], BF16, tag="wall",
                                name="wall")
            nc.sync.dma_start(wallt[:, bass.ds(0, GCOL)],
                              wall[:, bass.ds(0, GCOL)])
            augt = constp.tile([1, N], BF16, tag="aug", name="aug")
            nc.sync.dma_start(augt[:], aug[:])
            scbt = constp.tile([128, N], F32, tag="scb", name="scb")
            nc.sync.dma_start(scbt[:], scb[:])
            x2t = constp.tile([128, NTILES], F32, tag="x2", name="x2")
            nc.sync.dma_start(x2t[:], x2m[:])
            obbt = constp.tile([128, D], F32, tag="obb", name="obb")
            nc.sync.dma_start(obbt[:], obb[:])
            ones = constp.tile([1, N], BF16, tag="ones", name="ones")
            nc.gpsimd.memset(ones[:], 1.0)
            for g in range(1, NLOC // GEXP):
                nc.sync.dma_start(wallt[:, bass.ds(g * GCOL, GCOL)],
                                  wall[:, bass.ds(g * GCOL, GCOL)])

            acc = [constp.tile([128, D], F32, tag=f"acc{i}", name=f"acc{i}")
                   for i in range(NTILES)]
            attn = [constp.tile([128, N], F32, tag=f"attn{i}", name=f"attn{i}")
                    for i in range(NTILES)]

            partial = dramp.tile([BT, D], F32, tag="partial", name="partial")
            rs_out = [dramp.tile([(i1 - i0) * 128 // NCORES, D], F32,
                                 tag=f"rso{b}", name=f"rso{b}")
                      for b, (i0, i1) in enumerate(BLKS)]

            pe_prev = [None]  # PE total-order chain

            def emit_attention(i):
                xps = psmall.tile([128, N], F32, tag="xps", name="xps")
                for k in range(KCH):
                    pe_prev[0] = _mm(nc, xps[:], xth[k][:, ts(i, 128)],
                                     postt[k][:], k == 0, False, True,
                                     pe_prev[0])
                pe_prev[0] = _mm(nc, xps[:], ones[:], augt[:], False, True,
                                 True, pe_prev[0])
                dist = tmpp.tile([128, N], F32, tag="dist", name="dist")
                nc.scalar.activation(dist[:], xps[:], AF.Sqrt,
                                     bias=x2t[:, i:i + 1], scale=-2.0)
                nc.vector.tensor_scalar_add(dist[:], dist[:], 0.1)
                rec = tmpp.tile([128, N], F32, tag="rec", name="rec")
                nc.vector.reciprocal(rec[:], dist[:])
                nc.vector.tensor_mul(rec[:], rec[:], scbt[:])
                mx = statp.tile([128, 1], F32, tag="mx", name="mx")
                nc.vector.tensor_reduce(mx[:], rec[:], axis=mybir.AxisListType.X,
                                        op=mybir.AluOpType.max)
                negmx = statp.tile([128, 1], F32, tag="negmx", name="negmx")
                nc.vector.tensor_scalar_mul(negmx[:], mx[:], -1.0)
                ex = tmpp.tile([128, N], F32, tag="ex", name="ex")
                nc.scalar.activation(ex[:], rec[:], AF.Exp,
                                     bias=negmx[:], scale=1.0)
                sm = statp.tile([128, 1], F32, tag="sm", name="sm")
                nc.vector.tensor_reduce(sm[:], ex[:], axis=mybir.AxisListType.X,
                                        op=mybir.AluOpType.add)
                rsum = statp.tile([128, 1], F32, tag="rsum", name="rsum")
                nc.vector.reciprocal(rsum[:], sm[:])
                nc.vector.tensor_scalar_mul(attn[i][:], ex[:], rsum[:])

            emit_attention(0)
            emit_attention(1)

            for i in range(NTILES):
                for g in range(NLOC // GEXP):
                    pps = [pmain.tile([128, D], F32, tag="pm", name="pm")
                           for _ in range(GEXP)]
                    for k in range(KCH):
                        xap = xth[k][:, ts(i, 128)]
                        for j in range(GEXP):
                            nl = g * GEXP + j
                            wap = wallt[:, bass.ds((nl * KCH + k) * D, D)]
                            pe_prev[0] = _mm(
                                nc, pps[j][:], xap, wap,
                                k == 0, k == KCH - 1,
                                (not LDW_SKIP) or (j == 0), pe_prev[0])
                    for j in range(GEXP):
                        nl = g * GEXP + j
                        col = attn[i][:, nl:nl + 1]
                        if nl == 0:
                            nc.vector.scalar_tensor_tensor(
                                acc[i][:], pps[j][:], col, obbt[:],
                                op0=mybir.AluOpType.mult,
                                op1=mybir.AluOpType.add)
                        elif (nl + i) % 5 < 3:
                            nc.vector.scalar_tensor_tensor(
                                acc[i][:], pps[j][:], col, acc[i][:],
                                op0=mybir.AluOpType.mult,
                                op1=mybir.AluOpType.add)
                        else:
                            sc = tmpp.tile([128, D], F32, tag="sc", name="sc")
                            nc.scalar.activation(sc[:], pps[j][:], AF.Copy,
                                                 scale=col)
                            nc.gpsimd.tensor_add(acc[i][:], acc[i][:], sc[:])
                if i + 2 < NTILES:
                    emit_attention(i + 2)
                nc.sync.dma_start(partial[ts(i, 128), :], acc[i][:])
                for b, (i0, i1) in enumerate(BLKS):
                    if i == i1 - 1:
                        nc.gpsimd.collective_compute(
                            "ReduceScatter",
                            mybir.AluOpType.add,
                            replica_groups=[list(range(NCORES))],
                            ins=[partial[bass.ds(i0 * 128, (i1 - i0) * 128), :]],
                            outs=[rs_out[b][:]],
                        )

            yoff = 0
            for b, (i0, i1) in enumerate(BLKS):
                rows = (i1 - i0) * 128 // NCORES
                nc.sync.dma_start(y[bass.ds(yoff, rows), :], rs_out[b][:])
                yoff += rows

    nc.compile()
    return nc


def kernel(x, positions, scales, value_weight, out_W, out_b):
    global _PROGRAM
    if _PROGRAM is None:
        _PROGRAM = _build_program()
    nc = _PROGRAM

    X = np.ascontiguousarray(np.asarray(x, np.float32).reshape(BT, D))
    XTh = np.ascontiguousarray(X.T).astype(ml_dtypes.bfloat16)
    x2 = (X.astype(np.float64) ** 2).sum(1).astype(np.float32)
    x2m = np.ascontiguousarray(x2.reshape(NTILES, 128).T)  # [128, NTILES]
    pos = np.asarray(positions, np.float32)
    pn2 = (pos.astype(np.float64) ** 2).sum(1)           # (N,)
    sc = np.asarray(scales, np.float32)
    oW = np.asarray(out_W, np.float32)
    # fold the output projection into the expert weights (host, fp32)
    vw2 = np.asarray(value_weight, np.float32) @ oW.T    # (N, D, D)
    obb = np.tile(np.asarray(out_b, np.float32) / NCORES, (128, 1))

    in_maps = []
    for c in range(NCORES):
        mine = np.arange(c * NLOC, (c + 1) * NLOC)
        rest = np.delete(np.arange(N), mine)
        perm = np.concatenate([mine, rest])
        # wall[p, (n*KCH+k)*D + e] = vw2[mine[n], k*128+p, e]
        wl = np.ascontiguousarray(
            vw2[mine].reshape(NLOC, KCH, 128, D).transpose(2, 0, 1, 3)
            .reshape(128, NLOC * WCOL)).astype(ml_dtypes.bfloat16)
        in_maps.append({
            "xt_h": XTh,
            "x2m": x2m,
            "post": np.ascontiguousarray(pos[perm].T).astype(ml_dtypes.bfloat16),
            "aug": (-0.5 * pn2[perm]).astype(np.float32).astype(
                ml_dtypes.bfloat16).reshape(1, N),
            "scb": np.tile(sc[perm], (128, 1)).astype(np.float32),
            "wall": wl,
            "obb": obb,
        })

    trace = os.environ.get("BASS_KERNEL_TRACE", "0") == "1"
    res = run_bass_kernel_spmd(nc, in_maps, core_ids=list(range(NCORES)),
                               trace=trace)
    if trace:
        kernel.last_exec_time_ns = res.exec_time_ns
        kernel.last_trace = (res.instructions_and_trace or (None, None))[1]

    yfull = np.empty((BT, D), np.float32)
    for r in range(NCORES):
        yr = res.results[r]["y"]
        yoff = 0
        for (i0, i1) in BLKS:
            shard = (i1 - i0) * 128 // NCORES
            g0 = i0 * 128 + shard * r
            yfull[g0:g0 + shard] = yr[yoff:yoff + shard]
            yoff += shard
    return yfull.reshape(B, T, D)


# revision 10
# speedup vs baseline: 1.0189x; 1.0189x over previous
"""GrowingCrystalAttention Trainium2 kernel.

Expert-parallel over 8 NeuronCores: each core handles 16 of the 128
"neurons" (experts). Per core:
  - attention: xp = X @ posT (bf16 PE), dist/softmax on ACT+DVE in fp32
  - main contraction, tile-outer: for each of 16 bt tiles, P_n = X @ W'_n
    as bf16 matmuls where W'_n = W_n @ out_W.T is folded on the host,
    acc += attn[:, n] * P_n via DVE stt / ACT scale + GPSIMD add.
    Experts are processed in groups of 4 sharing the stationary X chunk;
    when LDW_SKIP is on, the 3 trailing matmuls of each (group, k) skip
    the redundant LDWEIGHTS (explicit PE-order chain keeps this sound).
  - out_b/8 is folded into the first drain of each tile, so after the
    ReduceScatter the shard is final; partials RS'd in 4 chunks, the
    last chunk tiny so only ~a collective floor is exposed at the end
  - all output copies (rs_out -> y) happen at the very end so they never
    block partial-DMA dispatch (that serialized the collectives before)
  - weights live resident in SBUF (8.4 MB bf16), loaded once in 4 DMAs

SPMD trick: every core runs the identical program; per-core inputs are
permuted so that attention columns 0..15 are always the core's own experts.
"""
import os
import sys

sys.path.insert(0, "/opt/trn_rl_repo")

import numpy as np
import ml_dtypes

import concourse.bass as bass
import concourse.mybir as mybir
import concourse.tile as tile
from concourse import bacc
from concourse.bass import ts
from concourse.bass_utils import run_bass_kernel_spmd
from concourse.tile_rust import add_dep_helper

AF = mybir.ActivationFunctionType
F32 = mybir.dt.float32
BF16 = mybir.dt.bfloat16

NCORES = 8
B, T, D = 4, 512, 512
N = 128
BT = B * T          # 2048
NLOC = N // NCORES  # 16
NTILES = BT // 128  # 16
KCH = D // 128      # 4
# bt-tile blocks; each ends with a ReduceScatter.  2 MB messages are the
# only size that gets the fast ENCD plan (~80 GB/s vs ~25 at <=1.5 MB).
BLKS = [(0, 8), (8, 16)]
GEXP = 4                      # experts per group (shared stationary)
WCOL = KCH * D                # weight columns per expert in wall
GCOL = GEXP * WCOL            # weight columns per DMA group
LDW_SKIP = True               # skip LDWEIGHTS on trailing group matmuls

_PROGRAM = None  # cached across kernel() calls


def _mm(nc, out, lhsT, rhs, start, stop, ldw, prev):
    """matmul with explicit ldweights control + PE-order chaining."""
    if ldw:
        inst = nc.tensor.matmul(out, lhsT, rhs, start=start, stop=stop)
    else:
        eng = nc.tensor
        ifmap_ap = eng.lower_ap(rhs.opt({0}), opt=False)
        weights_ap = eng.lower_ap(lhsT.opt({0}), opt=False,
                                  for_matmul_weights=True)
        out_ap = eng.lower_ap(out)
        inst = eng.add_instruction(mybir.InstMatmult(
            name=eng.bass.get_next_instruction_name(),
            replication_resolution=0, replication_shift_amnt=0,
            replication_num_rows=0,
            start_tensor_calc=start, stop_tensor_calc=stop,
            ins=[ifmap_ap, weights_ap], outs=[out_ap],
            tile_position=(0, 0),
            tile_size=(128, 128),
            ldweights=False,
        ))
    if prev is not None:
        add_dep_helper(inst.ins, prev.ins, sync=True, reason="pe-order")
    return inst


def _build_program():
    nc = bacc.Bacc("TRN2", target_bir_lowering=False, debug=False,
                   num_devices=NCORES)

    xt_h = nc.dram_tensor("xt_h", [D, BT], BF16, kind="ExternalInput").ap()
    x2m = nc.dram_tensor("x2m", [128, NTILES], F32, kind="ExternalInput").ap()
    post = nc.dram_tensor("post", [D, N], BF16, kind="ExternalInput").ap()
    aug = nc.dram_tensor("aug", [1, N], BF16, kind="ExternalInput").ap()
    scb = nc.dram_tensor("scb", [128, N], F32, kind="ExternalInput").ap()
    wall = nc.dram_tensor("wall", [128, NLOC * WCOL], BF16,
                          kind="ExternalInput").ap()
    obb = nc.dram_tensor("obb", [128, D], F32, kind="ExternalInput").ap()
    y = nc.dram_tensor("y", [BT // NCORES, D], F32, kind="ExternalOutput").ap()

    with tile.TileContext(nc) as tc:
        with tc.tile_pool(name="const", bufs=1) as constp, \
             tc.tile_pool(name="tmp", bufs=4) as tmpp, \
             tc.tile_pool(name="stat", bufs=6) as statp, \
             tc.tile_pool(name="pmain", bufs=6, space="PSUM") as pmain, \
             tc.tile_pool(name="psmall", bufs=2, space="PSUM") as psmall, \
             tc.tile_pool(name="dram", bufs=1, space="DRAM") as dramp:

            # ---- input DMAs on the sync queue, in priority order ----
            xth = [constp.tile([128, BT], BF16, tag=f"xth{k}", name=f"xth{k}")
                   for k in range(KCH)]
            for k in range(KCH):
                nc.sync.dma_start(xth[k][:], xt_h[ts(k, 128), :])
            postt = [constp.tile([128, N], BF16, tag=f"post{k}", name=f"post{k}")
                     for k in range(KCH)]
            for k in range(KCH):
                nc.sync.dma_start(postt[k][:], post[ts(k, 128), :])
            wallt = constp.tile([128, NLOC * WCOL], BF16, tag="wall",
                                name="wall")
            nc.sync.dma_start(wallt[:, bass.ds(0, GCOL)],
                              wall[:, bass.ds(0, GCOL)])
            augt = constp.tile([1, N], BF16, tag="aug", name="aug")
            nc.sync.dma_start(augt[:], aug[:])
            scbt = constp.tile([128, N], F32, tag="scb", name="scb")
            nc.sync.dma_start(scbt[:], scb[:])
            x2t = constp.tile([128, NTILES], F32, tag="x2", name="x2")
            nc.sync.dma_start(x2t[:], x2m[:])
            obbt = constp.tile([128, D], F32, tag="obb", name="obb")
            nc.sync.dma_start(obbt[:], obb[:])
            ones = constp.tile([1, N], BF16, tag="ones", name="ones")
            nc.gpsimd.memset(ones[:], 1.0)
            for g in range(1, NLOC // GEXP):
                nc.sync.dma_start(wallt[:, bass.ds(g * GCOL, GCOL)],
                                  wall[:, bass.ds(g * GCOL, GCOL)])

            acc = [constp.tile([128, D], F32, tag=f"acc{i}", name=f"acc{i}")
                   for i in range(NTILES)]
            attn = [constp.tile([128, N], F32, tag=f"attn{i}", name=f"attn{i}")
                    for i in range(NTILES)]

            partial = dramp.tile([BT, D], F32, tag="partial", name="partial")
            rs_out = [dramp.tile([(i1 - i0) * 128 // NCORES, D], F32,
                                 tag=f"rso{b}", name=f"rso{b}")
                      for b, (i0, i1) in enumerate(BLKS)]

            pe_prev = [None]  # PE total-order chain

            def emit_attention(i):
                xps = psmall.tile([128, N], F32, tag="xps", name="xps")
                for k in range(KCH):
                    pe_prev[0] = _mm(nc, xps[:], xth[k][:, ts(i, 128)],
                                     postt[k][:], k == 0, False, True,
                                     pe_prev[0])
                pe_prev[0] = _mm(nc, xps[:], ones[:], augt[:], False, True,
                                 True, pe_prev[0])
                dist = tmpp.tile([128, N], F32, tag="dist", name="dist")
                nc.scalar.activation(dist[:], xps[:], AF.Sqrt,
                                     bias=x2t[:, i:i + 1], scale=-2.0)
                nc.vector.tensor_scalar_add(dist[:], dist[:], 0.1)
                rec = tmpp.tile([128, N], F32, tag="rec", name="rec")
                nc.vector.reciprocal(rec[:], dist[:])
                nc.vector.tensor_mul(rec[:], rec[:], scbt[:])
                mx = statp.tile([128, 1], F32, tag="mx", name="mx")
                nc.vector.tensor_reduce(mx[:], rec[:], axis=mybir.AxisListType.X,
                                        op=mybir.AluOpType.max)
                negmx = statp.tile([128, 1], F32, tag="negmx", name="negmx")
                nc.vector.tensor_scalar_mul(negmx[:], mx[:], -1.0)
                ex = tmpp.tile([128, N], F32, tag="ex", name="ex")
                nc.scalar.activation(ex[:], rec[:], AF.Exp,
                                     bias=negmx[:], scale=1.0)
                sm = statp.tile([128, 1], F32, tag="sm", name="sm")
                nc.vector.tensor_reduce(sm[:], ex[:], axis=mybir.AxisListType.X,
                                        op=mybir.AluOpType.add)
                rsum = statp.tile([128, 1], F32, tag="rsum", name="rsum")
                nc.vector.reciprocal(rsum[:], sm[:])
                nc.vector.tensor_scalar_mul(attn[i][:], ex[:], rsum[:])

            emit_attention(0)
            emit_attention(1)

            for i in range(NTILES):
                for g in range(NLOC // GEXP):
                    pps = [pmain.tile([128, D], F32, tag="pm", name="pm")
                           for _ in range(GEXP)]
                    for k in range(KCH):
                        xap = xth[k][:, ts(i, 128)]
                        for j in range(GEXP):
                            nl = g * GEXP + j
                            wap = wallt[:, bass.ds((nl * KCH + k) * D, D)]
                            pe_prev[0] = _mm(
                                nc, pps[j][:], xap, wap,
                                k == 0, k == KCH - 1,
                                (not LDW_SKIP) or (j == 0), pe_prev[0])
                    for j in range(GEXP):
                        nl = g * GEXP + j
                        col = attn[i][:, nl:nl + 1]
                        # All drains on DVE: engine-serial, no cross-engine
                        # handoff latency in the acc chain (~8.8us/tile,
                        # comfortably under the ~14us of matmuls).
                        nc.vector.scalar_tensor_tensor(
                            acc[i][:], pps[j][:], col,
                            obbt[:] if nl == 0 else acc[i][:],
                            op0=mybir.AluOpType.mult,
                            op1=mybir.AluOpType.add)
                if i + 2 < NTILES:
                    emit_attention(i + 2)
                nc.sync.dma_start(partial[ts(i, 128), :], acc[i][:])
                for b, (i0, i1) in enumerate(BLKS):
                    if i == i1 - 1:
                        nc.gpsimd.collective_compute(
                            "ReduceScatter",
                            mybir.AluOpType.add,
                            replica_groups=[list(range(NCORES))],
                            ins=[partial[bass.ds(i0 * 128, (i1 - i0) * 128), :]],
                            outs=[rs_out[b][:]],
                        )

            yoff = 0
            for b, (i0, i1) in enumerate(BLKS):
                rows = (i1 - i0) * 128 // NCORES
                nc.sync.dma_start(y[bass.ds(yoff, rows), :], rs_out[b][:])
                yoff += rows

    nc.compile()
    return nc


def kernel(x, positions, scales, value_weight, out_W, out_b):
    global _PROGRAM
    if _PROGRAM is None:
        _PROGRAM = _build_program()
    nc = _PROGRAM

    X = np.ascontiguousarray(np.asarray(x, np.float32).reshape(BT, D))
    XTh = np.ascontiguousarray(X.T).astype(ml_dtypes.bfloat16)
    x2 = (X.astype(np.float64) ** 2).sum(1).astype(np.float32)
    x2m = np.ascontiguousarray(x2.reshape(NTILES, 128).T)  # [128, NTILES]
    pos = np.asarray(positions, np.float32)
    pn2 = (pos.astype(np.float64) ** 2).sum(1)           # (N,)
    sc = np.asarray(scales, np.float32)
    oW = np.asarray(out_W, np.float32)
    # fold the output projection into the expert weights (host, fp32)
    vw2 = np.asarray(value_weight, np.float32) @ oW.T    # (N, D, D)
    obb = np.tile(np.asarray(out_b, np.float32) / NCORES, (128, 1))

    in_maps = []
    for c in range(NCORES):
        mine = np.arange(c * NLOC, (c + 1) * NLOC)
        rest = np.delete(np.arange(N), mine)
        perm = np.concatenate([mine, rest])
        # wall[p, (n*KCH+k)*D + e] = vw2[mine[n], k*128+p, e]
        wl = np.ascontiguousarray(
            vw2[mine].reshape(NLOC, KCH, 128, D).transpose(2, 0, 1, 3)
            .reshape(128, NLOC * WCOL)).astype(ml_dtypes.bfloat16)
        in_maps.append({
            "xt_h": XTh,
            "x2m": x2m,
            "post": np.ascontiguousarray(pos[perm].T).astype(ml_dtypes.bfloat16),
            "aug": (-0.5 * pn2[perm]).astype(np.float32).astype(
                ml_dtypes.bfloat16).reshape(1, N),
            "scb": np.tile(sc[perm], (128, 1)).astype(np.float32),
            "wall": wl,
            "obb": obb,
        })

    trace = os.environ.get("BASS_KERNEL_TRACE", "0") == "1"
    res = run_bass_kernel_spmd(nc, in_maps, core_ids=list(range(NCORES)),
                               trace=trace)
    if trace:
        kernel.last_exec_time_ns = res.exec_time_ns
        kernel.last_trace = (res.instructions_and_trace or (None, None))[1]

    yfull = np.empty((BT, D), np.float32)
    for r in range(NCORES):
        yr = res.results[r]["y"]
        yoff = 0
        for (i0, i1) in BLKS:
            shard = (i1 - i0) * 128 // NCORES
            g0 = i0 * 128 + shard * r
            yfull[g0:g0 + shard] = yr[yoff:yoff + shard]
            yoff += shard
    return yfull.reshape(B, T, D)


# revision 21
# speedup vs baseline: 1.1307x; 1.1098x over previous
"""GrowingCrystalAttention Trainium2 kernel.

Expert-parallel over 8 NeuronCores: each core handles 16 of the 128
"neurons" (experts). Per core:
  - attention: xp = X @ posT (bf16 PE), dist/softmax on ACT+DVE in fp32
  - main contraction, tile-outer: for each of 16 bt tiles, P_n = X @ W'_n
    as bf16 matmuls where W'_n = W_n @ out_W.T is folded on the host,
    acc += attn[:, n] * P_n via DVE stt / ACT scale + GPSIMD add.
    Experts are processed in groups of 4 sharing the stationary X chunk;
    when LDW_SKIP is on, the 3 trailing matmuls of each (group, k) skip
    the redundant LDWEIGHTS (explicit PE-order chain keeps this sound).
  - out_b/8 is folded into the first drain of each tile, so after the
    ReduceScatter the shard is final; partials RS'd in 4 chunks, the
    last chunk tiny so only ~a collective floor is exposed at the end
  - all output copies (rs_out -> y) happen at the very end so they never
    block partial-DMA dispatch (that serialized the collectives before)
  - weights live resident in SBUF (8.4 MB bf16), loaded once in 4 DMAs

SPMD trick: every core runs the identical program; per-core inputs are
permuted so that attention columns 0..15 are always the core's own experts.
"""
import os
import sys

sys.path.insert(0, "/opt/trn_rl_repo")

import numpy as np
import ml_dtypes

import concourse.bass as bass
import concourse.mybir as mybir
import concourse.tile as tile
from concourse import bacc
from concourse.bass import ts
from concourse.bass_utils import run_bass_kernel_spmd
from concourse.tile_rust import add_dep_helper

AF = mybir.ActivationFunctionType
F32 = mybir.dt.float32
BF16 = mybir.dt.bfloat16

NCORES = 8
B, T, D = 4, 512, 512
N = 128
BT = B * T          # 2048
NLOC = N // NCORES  # 16
NTILES = BT // 128  # 16
KCH = D // 128      # 4
# bt-tile blocks; each ends with a ReduceScatter.  Big messages get a
# much better ENCD plan; the mid chunk still hides under compute and
# only the small last chunk's collective is exposed at the end.
BLKS = [(0, 8), (8, 14), (14, 16)]
GEXP = 2                      # experts per group (shared stationary)
WCOL = KCH * D                # weight columns per expert in wall
GCOL = GEXP * WCOL            # weight columns per DMA group
LDW_SKIP = True               # skip LDWEIGHTS on trailing group matmuls

_PROGRAM = None  # cached across kernel() calls


def _mm(nc, out, lhsT, rhs, start, stop, ldw, prev):
    """matmul with explicit ldweights control + PE-order chaining."""
    if ldw:
        inst = nc.tensor.matmul(out, lhsT, rhs, start=start, stop=stop)
    else:
        eng = nc.tensor
        ifmap_ap = eng.lower_ap(rhs.opt({0}), opt=False)
        weights_ap = eng.lower_ap(lhsT.opt({0}), opt=False,
                                  for_matmul_weights=True)
        out_ap = eng.lower_ap(out)
        inst = eng.add_instruction(mybir.InstMatmult(
            name=eng.bass.get_next_instruction_name(),
            replication_resolution=0, replication_shift_amnt=0,
            replication_num_rows=0,
            start_tensor_calc=start, stop_tensor_calc=stop,
            ins=[ifmap_ap, weights_ap], outs=[out_ap],
            tile_position=(0, 0),
            tile_size=(128, 128),
            ldweights=False,
        ))
    if prev is not None:
        add_dep_helper(inst.ins, prev.ins, sync=True, reason="pe-order")
    return inst


def _build_program():
    nc = bacc.Bacc("TRN2", target_bir_lowering=False, debug=False,
                   num_devices=NCORES)

    xt_h = nc.dram_tensor("xt_h", [D, BT], BF16, kind="ExternalInput").ap()
    x2m = nc.dram_tensor("x2m", [128, NTILES], F32, kind="ExternalInput").ap()
    post = nc.dram_tensor("post", [D, N], BF16, kind="ExternalInput").ap()
    aug = nc.dram_tensor("aug", [1, N], BF16, kind="ExternalInput").ap()
    scb = nc.dram_tensor("scb", [128, N], F32, kind="ExternalInput").ap()
    wall = nc.dram_tensor("wall", [128, NLOC * WCOL], BF16,
                          kind="ExternalInput").ap()
    obb = nc.dram_tensor("obb", [128, D], F32, kind="ExternalInput").ap()
    y = nc.dram_tensor("y", [BT // NCORES, D], F32, kind="ExternalOutput").ap()

    with tile.TileContext(nc) as tc:
        with tc.tile_pool(name="const", bufs=1) as constp, \
             tc.tile_pool(name="tmp", bufs=4) as tmpp, \
             tc.tile_pool(name="stat", bufs=6) as statp, \
             tc.tile_pool(name="pmain", bufs=6, space="PSUM") as pmain, \
             tc.tile_pool(name="psmall", bufs=2, space="PSUM") as psmall, \
             tc.tile_pool(name="dram", bufs=1, space="DRAM") as dramp:

            # ---- input DMAs on the sync queue, in priority order ----
            xth = [constp.tile([128, BT], BF16, tag=f"xth{k}", name=f"xth{k}")
                   for k in range(KCH)]
            for k in range(KCH):
                nc.sync.dma_start(xth[k][:], xt_h[ts(k, 128), :])
            postt = [constp.tile([128, N], BF16, tag=f"post{k}", name=f"post{k}")
                     for k in range(KCH)]
            for k in range(KCH):
                nc.sync.dma_start(postt[k][:], post[ts(k, 128), :])
            wallt = constp.tile([128, NLOC * WCOL], BF16, tag="wall",
                                name="wall")
            nc.sync.dma_start(wallt[:, bass.ds(0, GCOL)],
                              wall[:, bass.ds(0, GCOL)])
            augt = constp.tile([1, N], BF16, tag="aug", name="aug")
            nc.sync.dma_start(augt[:], aug[:])
            scbt = constp.tile([128, N], F32, tag="scb", name="scb")
            nc.sync.dma_start(scbt[:], scb[:])
            x2t = constp.tile([128, NTILES], F32, tag="x2", name="x2")
            nc.sync.dma_start(x2t[:], x2m[:])
            obbt = constp.tile([128, D], F32, tag="obb", name="obb")
            nc.sync.dma_start(obbt[:], obb[:])
            ones = constp.tile([1, N], BF16, tag="ones", name="ones")
            nc.gpsimd.memset(ones[:], 1.0)
            for g in range(1, NLOC // GEXP):
                nc.sync.dma_start(wallt[:, bass.ds(g * GCOL, GCOL)],
                                  wall[:, bass.ds(g * GCOL, GCOL)])

            attn = [constp.tile([128, N], F32, tag=f"attn{i}", name=f"attn{i}")
                    for i in range(NTILES)]

            partial = dramp.tile([BT, D], F32, tag="partial", name="partial")
            rs_out = [dramp.tile([(i1 - i0) * 128 // NCORES, D], F32,
                                 tag=f"rso{b}", name=f"rso{b}")
                      for b, (i0, i1) in enumerate(BLKS)]

            pe_prev = [None]  # PE total-order chain

            def emit_attention(i):
                xps = psmall.tile([128, N], F32, tag="xps", name="xps")
                for k in range(KCH):
                    pe_prev[0] = _mm(nc, xps[:], xth[k][:, ts(i, 128)],
                                     postt[k][:], k == 0, False, True,
                                     pe_prev[0])
                pe_prev[0] = _mm(nc, xps[:], ones[:], augt[:], False, True,
                                 True, pe_prev[0])
                # Softmax bulk runs on the otherwise-idle GPSIMD so the DVE
                # queue carries only drains (jitter there stalls the PE via
                # late PSUM frees).  DVE keeps the two reciprocals (DVE-only).
                dist = tmpp.tile([128, N], F32, tag="dist", name="dist")
                nc.scalar.activation(dist[:], xps[:], AF.Sqrt,
                                     bias=x2t[:, i:i + 1], scale=-2.0)
                nc.gpsimd.tensor_scalar_add(dist[:], dist[:], 0.1)
                rec = tmpp.tile([128, N], F32, tag="rec", name="rec")
                nc.vector.reciprocal(rec[:], dist[:])
                nc.gpsimd.tensor_mul(rec[:], rec[:], scbt[:])
                mx = statp.tile([128, 1], F32, tag="mx", name="mx")
                nc.vector.tensor_reduce(mx[:], rec[:], axis=mybir.AxisListType.X,
                                        op=mybir.AluOpType.max)
                negmx = statp.tile([128, 1], F32, tag="negmx", name="negmx")
                nc.gpsimd.tensor_scalar_mul(negmx[:], mx[:], -1.0)
                ex = tmpp.tile([128, N], F32, tag="ex", name="ex")
                nc.scalar.activation(ex[:], rec[:], AF.Exp,
                                     bias=negmx[:], scale=1.0)
                sm = statp.tile([128, 1], F32, tag="sm", name="sm")
                nc.vector.tensor_reduce(sm[:], ex[:], axis=mybir.AxisListType.X,
                                        op=mybir.AluOpType.add)
                rsum = statp.tile([128, 1], F32, tag="rsum", name="rsum")
                nc.vector.reciprocal(rsum[:], sm[:])
                nc.gpsimd.tensor_scalar_mul(attn[i][:], ex[:], rsum[:])

            emit_attention(0)
            emit_attention(1)

            for i in range(NTILES):
                acc = constp.tile([128, D], F32, tag=f"acc{i}",
                                  name=f"acc{i}")
                for g in range(NLOC // GEXP):
                    pps = [pmain.tile([128, D], F32, tag="pm", name="pm")
                           for _ in range(GEXP)]
                    for k in range(KCH):
                        xap = xth[k][:, ts(i, 128)]
                        for j in range(GEXP):
                            nl = g * GEXP + j
                            wap = wallt[:, bass.ds((nl * KCH + k) * D, D)]
                            pe_prev[0] = _mm(
                                nc, pps[j][:], xap, wap,
                                k == 0, k == KCH - 1,
                                (not LDW_SKIP) or (j == 0), pe_prev[0])
                    for j in range(GEXP):
                        nl = g * GEXP + j
                        col = attn[i][:, nl:nl + 1]
                        nc.vector.scalar_tensor_tensor(
                            acc[:], pps[j][:], col,
                            obbt[:] if nl == 0 else acc[:],
                            op0=mybir.AluOpType.mult,
                            op1=mybir.AluOpType.add)
                if i + 2 < NTILES:
                    emit_attention(i + 2)
                nc.sync.dma_start(partial[ts(i, 128), :], acc[:])
                for b, (i0, i1) in enumerate(BLKS):
                    if i == i1 - 1:
                        nc.gpsimd.collective_compute(
                            "ReduceScatter",
                            mybir.AluOpType.add,
                            replica_groups=[list(range(NCORES))],
                            ins=[partial[bass.ds(i0 * 128, (i1 - i0) * 128), :]],
                            outs=[rs_out[b][:]],
                        )

            yoff = 0
            for b, (i0, i1) in enumerate(BLKS):
                rows = (i1 - i0) * 128 // NCORES
                nc.sync.dma_start(y[bass.ds(yoff, rows), :], rs_out[b][:])
                yoff += rows

    nc.compile()
    return nc


def kernel(x, positions, scales, value_weight, out_W, out_b):
    global _PROGRAM
    if _PROGRAM is None:
        _PROGRAM = _build_program()
    nc = _PROGRAM

    X = np.ascontiguousarray(np.asarray(x, np.float32).reshape(BT, D))
    XTh = np.ascontiguousarray(X.T).astype(ml_dtypes.bfloat16)
    x2 = (X.astype(np.float64) ** 2).sum(1).astype(np.float32)
    x2m = np.ascontiguousarray(x2.reshape(NTILES, 128).T)  # [128, NTILES]
    pos = np.asarray(positions, np.float32)
    pn2 = (pos.astype(np.float64) ** 2).sum(1)           # (N,)
    sc = np.asarray(scales, np.float32)
    oW = np.asarray(out_W, np.float32)
    # fold the output projection into the expert weights (host, fp32)
    vw2 = np.asarray(value_weight, np.float32) @ oW.T    # (N, D, D)
    obb = np.tile(np.asarray(out_b, np.float32) / NCORES, (128, 1))

    in_maps = []
    for c in range(NCORES):
        mine = np.arange(c * NLOC, (c + 1) * NLOC)
        rest = np.delete(np.arange(N), mine)
        perm = np.concatenate([mine, rest])
        # wall[p, (n*KCH+k)*D + e] = vw2[mine[n], k*128+p, e]
        wl = np.ascontiguousarray(
            vw2[mine].reshape(NLOC, KCH, 128, D).transpose(2, 0, 1, 3)
            .reshape(128, NLOC * WCOL)).astype(ml_dtypes.bfloat16)
        in_maps.append({
            "xt_h": XTh,
            "x2m": x2m,
            "post": np.ascontiguousarray(pos[perm].T).astype(ml_dtypes.bfloat16),
            "aug": (-0.5 * pn2[perm]).astype(np.float32).astype(
                ml_dtypes.bfloat16).reshape(1, N),
            "scb": np.tile(sc[perm], (128, 1)).astype(np.float32),
            "wall": wl,
            "obb": obb,
        })

    trace = os.environ.get("BASS_KERNEL_TRACE", "0") == "1"
    res = run_bass_kernel_spmd(nc, in_maps, core_ids=list(range(NCORES)),
                               trace=trace)
    if trace:
        kernel.last_exec_time_ns = res.exec_time_ns
        kernel.last_trace = (res.instructions_and_trace or (None, None))[1]

    yfull = np.empty((BT, D), np.float32)
    for r in range(NCORES):
        yr = res.results[r]["y"]
        yoff = 0
        for (i0, i1) in BLKS:
            shard = (i1 - i0) * 128 // NCORES
            g0 = i0 * 128 + shard * r
            yfull[g0:g0 + shard] = yr[yoff:yoff + shard]
            yoff += shard
    return yfull.reshape(B, T, D)
